# revision 49
# baseline (speedup 1.0000x reference)
"""Trainium2 Bass kernel for nn_DeterministicEgnnPolicy (EGNN message passing).

Strategy (per sharding hint): shard the 1024 independent 32-node graphs
across 8 NeuronCores (128 graphs/core). On each core the fully-connected
edge structure is computed densely as all-pairs 32x32 blocks:

- "feature-major" edge tensors [128 = 2 graph-halves x 64 features,
  (i, j)] drive the edge-MLP matmuls (bf16 operands, fp32 PSUM).
- Accuracy scheme (validated vs fp32 reference on CPU): h state is fp32
  master with bf16 hi/lo split shadows; per-node A = Wi.T h, B = Wj.T h are
  precomputed in split precision and broadcast into edge tensors via
  identity-stationary matmuls, so the big edge GEMMs see once-rounded bf16.
- Per-edge radial/edge_attr live in an "EW" tile [128 = (chunk, row),
  (u, i, j)] computed on-chip by DVE broadcast ops from per-graph coords
  (XI tiles, filled by selector matmuls) - no layout-conversion DMAs.
- Edge loop is software-pipelined (lag emission) across 4 PSUM stage banks.

Graph indexing on a core: g = gb*4 + gm, gb in [0,32), gm in [0,4).
half = gb//16 (feature partitions 64*half..64*half+63).
node free index (per half): n' = gb_l*128 + gm*32 + i, gb_l = gb%16.
chunk c = gmp*16 + gb_l, rows (0: rad h0, 1: ea h0, 2: rad h1, 3: ea h1).
"""

import numpy as np

N_AGENTS = 32
BATCH = 1024
H = 64
L = 4
INV = 16
DEG = float(N_AGENTS - 1)
NCORES = 8
G_CORE = BATCH // NCORES          # 128 graphs per core
NGB = G_CORE // 4                 # 32 gb blocks per core
NGBL = NGB // 2                   # 16 per half
NNODE = NGBL * 128                # 2048 node free dim (per half)
NODES_CORE = G_CORE * N_AGENTS    # 4096

_BUILD_CACHE = {}


# ----------------------------------------------------------------------------
# Host-side packing (pure layout permutation / weight arrangement)
# ----------------------------------------------------------------------------

def _bd(w):
    """64x64 block-diagonal lhsT [128,128] from w [64,64] (or [k,64])."""
    k = w.shape[0]
    out = np.zeros((128, 128), np.float32)
    out[0:k, 0:64] = w
    out[64:64 + k, 64:128] = w
    return out


def _bd_rep(wcol):
    """Replicating lhsT: out[64h+f, 64h+f'] = wcol[f] for all f'."""
    out = np.zeros((128, 128), np.float32)
    col = wcol.reshape(64, 1)
    out[0:64, 0:64] = np.repeat(col, 64, axis=1)
    out[64:128, 64:128] = np.repeat(col, 64, axis=1)
    return out


BNAMES = ["emb"]
for _l in range(L):
    BNAMES += [f"{nm}{_l}" for nm in
               ("Wi", "Wj", "Wsc", "We2", "Wc1",
                "Wc2", "Wv1", "Wv2", "Wn1t", "Wn1b", "Wn1d", "Wn2")]
NWB = len(BNAMES)
FNAMES = ["ident", "delta", "lsel"]


def _pack_weights(inp):
    """Build wpackb [128, NWB*128] (bf16), wpackf [128, 3*128] (f32),
    biaspack [128, NBIAS] (f32)."""
    import ml_dtypes

    btiles = {}

    def add(name, arr):
        t = np.zeros((128, 128), np.float32)
        t[:arr.shape[0], :arr.shape[1]] = arr
        btiles[name] = t

    emb = np.zeros((128, 128), np.float32)
    emb[0:INV, 0:64] = inp["emb_W"]
    emb[64:64 + INV, 64:128] = inp["emb_W"]
    add("emb", emb)

    for l in range(L):
        We1 = np.asarray(inp["We1"][l], np.float32)   # [130, 64]
        add(f"Wi{l}", _bd(We1[0:64]))
        add(f"Wj{l}", _bd(We1[64:128]))
        wsc = np.zeros((4, 128), np.float32)
        wsc[0, 0:64] = We1[128]      # radial, half0
        wsc[1, 0:64] = We1[129]      # edge_attr, half0
        wsc[2, 64:128] = We1[128]
        wsc[3, 64:128] = We1[129]
        add(f"Wsc{l}", wsc)
        add(f"We2{l}", _bd(inp["We2"][l]))
        add(f"Wc1{l}", _bd(inp["Wc1"][l]))
        add(f"Wc2{l}", _bd_rep(np.asarray(inp["Wc2"][l], np.float32)[:, 0]))
        add(f"Wv1{l}", _bd(inp["Wv1"][l]))
        add(f"Wv2{l}", _bd_rep(np.asarray(inp["Wv2"][l], np.float32)[:, 0]))
        Wn1 = np.asarray(inp["Wn1"][l], np.float32)   # [128, 64]
        add(f"Wn1t{l}", _bd(Wn1[0:64]))
        add(f"Wn1b{l}", _bd(Wn1[64:128]))
        add(f"Wn1d{l}", _bd(-Wn1[64:128]))
        add(f"Wn2{l}", _bd(inp["Wn2"][l]))

    wpackb = np.concatenate([btiles[n] for n in BNAMES], axis=1).astype(
        np.float16)

    ftiles = {}
    ftiles["ident"] = np.eye(128, dtype=np.float32)
    delta = np.zeros((128, 128), np.float32)
    for gm in range(4):
        delta[gm, gm * 32:(gm + 1) * 32] = 1.0
    ftiles["delta"] = delta
    # Lsel[k, gb_l*4 + row]: row0 -> cur x of gb_l (k=gb_l), row1 -> x0 of
    # gb_l (k=32+gb_l), row2 -> cur x of gb_l+16, row3 -> x0 of gb_l+16.
    lsel = np.zeros((128, 128), np.float32)
    for gb_l in range(NGBL):
        lsel[gb_l, gb_l * 4 + 0] = 1.0
        lsel[32 + gb_l, gb_l * 4 + 1] = 1.0
        lsel[gb_l + 16, gb_l * 4 + 2] = 1.0
        lsel[32 + gb_l + 16, gb_l * 4 + 3] = 1.0
    ftiles["lsel"] = lsel
    wpackf = np.concatenate([ftiles[n] for n in FNAMES], axis=1)

    bias_cols = []
    for l in range(L):
        for nm in ("be1", "be2", "bc1", "bv1", "bn1", "bn2"):
            bias_cols.append(np.tile(np.asarray(inp[nm][l]).reshape(-1), 2))
        for nm in ("bv2", "bc2"):
            bias_cols.append(np.full(128, float(np.asarray(inp[nm][l]).reshape(-1)[0]),
                                     np.float32))
    bias_cols.append(np.tile(np.asarray(inp["emb_b"]).reshape(-1), 2))
    biaspack = np.stack(bias_cols, axis=1).astype(np.float32)  # [128, NB]
    return wpackb, wpackf, biaspack


def _arrange_inputs(obs_slice):
    """Per-core obs slice [4096, 20] -> invT [128, 2048] fp16,
    locvel [128, 128] f32."""
    obs3 = obs_slice.reshape(NGB, 128, 20)          # [gb, (gm,i), col]
    invT = np.zeros((128, NNODE), np.float32)
    inv_half0 = obs3[0:NGBL, :, 0:INV]              # [16, 128, 16]
    inv_half1 = obs3[NGBL:NGB, :, 0:INV]
    invT[0:INV, :] = np.transpose(inv_half0, (2, 0, 1)).reshape(INV, NNODE)
    invT[64:64 + INV, :] = np.transpose(inv_half1, (2, 0, 1)).reshape(INV, NNODE)
    locvel = np.ascontiguousarray(
        np.transpose(obs3[:, :, INV:INV + 4], (1, 0, 2)).reshape(128, NGB * 4)
    ).astype(np.float32)
    return invT.astype(np.float16), locvel


def _unarrange_output(outP):
    """outP [128, 64] -> [4096, 2] (n = gb*128 + p)."""
    return np.ascontiguousarray(
        outP.reshape(128, NGB, 2).transpose(1, 0, 2).reshape(NODES_CORE, 2)
    )


def make_in_maps(inp):
    wpackb, wpackf, biaspack = _pack_weights(inp)
    obs = np.asarray(inp["obs"], np.float32)
    in_maps = []
    for c in range(NCORES):
        invT, locvel = _arrange_inputs(obs[c * NODES_CORE:(c + 1) * NODES_CORE])
        in_maps.append({"invT": invT, "locvel": locvel,
                        "wpackb": wpackb, "wpackf": wpackf,
                        "biaspack": biaspack})
    return in_maps


# ----------------------------------------------------------------------------
# Device kernel builder
# ----------------------------------------------------------------------------

def build(scale0, scale1, mean0, mean1):
    import concourse.bacc as bacc
    import concourse.tile as tile
    import concourse.mybir as mybir
    from contextlib import ExitStack

    F32 = mybir.dt.float32
    BF16 = mybir.dt.float16
    AT = mybir.AluOpType
    ACTF = mybir.ActivationFunctionType

    nc = bacc.Bacc("TRN2", target_bir_lowering=False, debug=False)

    invT_d = nc.dram_tensor("invT", [128, NNODE], BF16, kind="ExternalInput")
    locvel_d = nc.dram_tensor("locvel", [128, NGB * 4], F32, kind="ExternalInput")
    wpackb_d = nc.dram_tensor("wpackb", [128, NWB * 128], BF16, kind="ExternalInput")
    wpackf_d = nc.dram_tensor("wpackf", [128, len(FNAMES) * 128], F32,
                              kind="ExternalInput")
    NBIAS = 8 * L + 1
    bias_d = nc.dram_tensor("biaspack", [128, NBIAS], F32, kind="ExternalInput")
    out_d = nc.dram_tensor("out", [128, NGB * 2], F32, kind="ExternalOutput")

    widx = {n: i for i, n in enumerate(BNAMES)}
    fidx = {n: i for i, n in enumerate(FNAMES)}
    bidx = {}
    _bi = 0
    for l in range(L):
        for nm in ("be1", "be2", "bc1", "bv1", "bn1", "bn2", "bv2", "bc2"):
            bidx[f"{nm}{l}"] = _bi
            _bi += 1
    bidx["embb"] = _bi

    with tile.TileContext(nc) as tc, ExitStack() as ctx:
        st = ctx.enter_context(tc.tile_pool(name="static", bufs=1))
        eA = ctx.enter_context(tc.tile_pool(name="eA", bufs=2))
        eR = ctx.enter_context(tc.tile_pool(name="eR", bufs=3))
        mx = ctx.enter_context(tc.tile_pool(name="mx", bufs=1))
        ps = ctx.enter_context(tc.tile_pool(name="ps", bufs=1, space="PSUM"))

        # ---- static loads ----
        wsb = st.tile([128, NWB * 128], BF16)
        nc.sync.dma_start(wsb[:], wpackb_d.ap())
        wsf = st.tile([128, len(FNAMES) * 128], F32)
        nc.sync.dma_start(wsf[:], wpackf_d.ap())
        bsb = st.tile([128, NBIAS], F32)
        nc.sync.dma_start(bsb[:], bias_d.ap())
        invT = st.tile([128, NNODE], BF16)
        nc.sync.dma_start(invT[:], invT_d.ap())
        locvel = st.tile([128, NGB * 4], F32)
        nc.sync.dma_start(locvel[:], locvel_d.ap())

        def W(name):
            return wsb[:, widx[name] * 128:(widx[name] + 1) * 128]

        def Wf(name):
            return wsf[:, fidx[name] * 128:(fidx[name] + 1) * 128]

        def Bia(name):
            return bsb[:, bidx[name]:bidx[name] + 1]

        ident = Wf("ident")
        delta4 = Wf("delta")[0:4, :]
        lsel = Wf("lsel")[0:64, 0:64]

        # ---- persistent state ----
        hF = st.tile([128, NNODE], F32)
        h16 = st.tile([128, NNODE], BF16)
        magg = st.tile([128, NNODE], BF16)
        mdiag = st.tile([128, NNODE], BF16)
        smat = st.tile([128, 1024], F32)
        rad = st.tile([128, 1024], F32)
        dx = st.tile([128, 1024], F32)
        dy = st.tile([128, 1024], F32)
        EW = st.tile([128, 2048], BF16)
        XIx = st.tile([128, 64], F32)
        XIy = st.tile([128, 64], F32)
        LTx = st.tile([64, 128], F32)
        LTy = st.tile([64, 128], F32)
        locx = st.tile([128, NGB], F32)
        locy = st.tile([128, NGB], F32)
        velx = st.tile([128, NGB], F32)
        vely = st.tile([128, NGB], F32)
        phiPA = st.tile([128, NGB], F32)
        phiPB = st.tile([128, NGB], F32)
        hv1 = st.tile([128, NNODE], BF16)
        phirep = st.tile([128, NNODE], F32)
        T4x = st.tile([4, 1024], F32)
        T4y = st.tile([4, 1024], F32)
        outP = st.tile([128, NGB * 2], F32)

        lv = locvel[:].rearrange("p (gb c) -> p gb c", c=4)
        nc.vector.tensor_copy(locx[:], lv[:, :, 0])
        nc.vector.tensor_copy(locy[:], lv[:, :, 1])
        nc.vector.tensor_copy(velx[:], lv[:, :, 2])
        nc.vector.tensor_copy(vely[:], lv[:, :, 3])

        def heat(lhsT_ap, rhs_ap, n=14):
            hp = ps.tile([128, 512], F32, tag="ps2")
            for _ in range(n):
                nc.tensor.matmul(hp[:], lhsT_ap, rhs_ap, start=True, stop=True)

        # ---- embedding: h0 = inv @ emb_W + emb_b ----
        heat(W("emb"), invT[:, 0:512])
        for u in range(NNODE // 1024):
            pse = ps.tile([128, 1024], F32, tag="ps1")
            sl = slice(u * 1024, (u + 1) * 1024)
            for k in range(2):
                ksl = slice(u * 1024 + k * 512, u * 1024 + (k + 1) * 512)
                osl = slice(k * 512, (k + 1) * 512)
                nc.tensor.matmul(pse[:, osl], W("emb"), invT[:, ksl],
                                 start=True, stop=True)
            nc.vector.tensor_scalar_add(hF[:, sl], pse[:], Bia("embb"))

        def h_shadows(sl):
            nc.vector.tensor_copy(h16[:, sl], hF[:, sl])

        for u in range(2):
            h_shadows(slice(u * 1024, (u + 1) * 1024))

        def radial_part(first):
            """lxT/lyT -> LT tiles, T4s, dx, dy, rad from current locx/locy."""
            for (lP, LT) in ((locx, LTx), (locy, LTy)):
                pst = ps.tile([32, 128], F32, tag="ps3")
                nc.tensor.transpose(pst[:], lP[:], ident)
                nc.vector.tensor_copy(LT[0:32, :], pst[:])
                if first:
                    nc.vector.tensor_copy(LT[32:64, :], pst[:])
            for (LT, T4) in ((LTx, T4x), (LTy, T4y)):
                for gm in range(4):
                    nc.sync.dma_start(
                        T4[gm:gm + 1, :].rearrange("p (gb j) -> p gb j", j=32),
                        LT[0:32, gm * 32:(gm + 1) * 32])
            for (T4, lP, dT) in ((T4x, locx, dx), (T4y, locy, dy)):
                pss = ps.tile([128, 1024], F32, tag="ps2")
                for k in range(2):
                    nc.tensor.matmul(pss[:, k * 512:(k + 1) * 512], delta4,
                                     T4[:, k * 512:(k + 1) * 512],
                                     start=True, stop=True)
                bc = lP[:].unsqueeze(2).broadcast_to([128, NGB, 32])
                nc.vector.tensor_tensor(
                    dT[:].rearrange("p (gb j) -> p gb j", j=32), bc,
                    pss[:].rearrange("p (gb j) -> p gb j", j=32), op=AT.subtract)
            t2 = mx.tile([128, 1024], F32, tag="mx_t2")
            nc.vector.tensor_tensor(rad[:], dx[:], dx[:], op=AT.mult)
            nc.vector.tensor_tensor(t2[:], dy[:], dy[:], op=AT.mult)
            nc.vector.tensor_tensor(rad[:], rad[:], t2[:], op=AT.add)

        def xi_ew_part():
            """XI tiles from LT via selector matmuls, then EW = diff^2 sums."""
            for (LT, XI) in ((LTx, XIx), (LTy, XIy)):
                for gmp in range(2):
                    psxi = ps.tile([64, 64], F32, tag="ps4")
                    nc.tensor.matmul(psxi[:], lsel,
                                     LT[0:64, gmp * 64:(gmp + 1) * 64],
                                     start=True, stop=True)
                    nc.vector.tensor_copy(XI[gmp * 64:(gmp + 1) * 64, :], psxi[:])
            dxE = mx.tile([128, 2048], F32, tag="mx_dxE")
            dyE = mx.tile([128, 2048], F32, tag="mx_dyE")
            for (XI, dE) in ((XIx, dxE), (XIy, dyE)):
                x3 = XI[:].rearrange("p (u i) -> p u i", i=32)
                bi = x3.unsqueeze(3).broadcast_to([128, 2, 32, 32])
                bj = x3.unsqueeze(2).broadcast_to([128, 2, 32, 32])
                nc.gpsimd.tensor_tensor(
                    dE[:].rearrange("p (u i j) -> p u i j", i=32, j=32),
                    bi, bj, op=AT.subtract)
            sq = mx.tile([128, 2048], F32, tag="mx_sqE")
            nc.gpsimd.tensor_tensor(sq[:], dxE[:], dxE[:], op=AT.mult)
            sq2 = mx.tile([128, 2048], F32, tag="mx_sqE2")
            nc.gpsimd.tensor_tensor(sq2[:], dyE[:], dyE[:], op=AT.mult)
            with nc.allow_low_precision(reason="fp16 EW radial"):
                nc.gpsimd.tensor_tensor(EW[:], sq[:], sq2[:], op=AT.add)

        radial_part(first=True)
        xi_ew_part()

        def ab_node_phase(l):
            phiP = phiPA if l % 2 == 0 else phiPB
            # ---- node phase: phi = silu(h@Wv1+bv1)@Wv2 + bv2 -> phiP ----
            for u in range(NNODE // 1024):
                sl = slice(u * 1024, (u + 1) * 1024)
                psv = ps.tile([128, 1024], F32, tag="ps2")
                for k in range(2):
                    ksl = slice(u * 1024 + k * 512, u * 1024 + (k + 1) * 512)
                    osl = slice(k * 512, (k + 1) * 512)
                    nc.tensor.matmul(psv[:, osl], W(f"Wv1{l}"), h16[:, ksl],
                                     start=True, stop=True)
                nc.scalar.activation(hv1[:, sl], psv[:], ACTF.Silu,
                                     bias=Bia(f"bv1{l}"))
                psv2 = ps.tile([128, 1024], F32, tag="ps3")
                for k in range(2):
                    ksl = slice(u * 1024 + k * 512, u * 1024 + (k + 1) * 512)
                    nc.tensor.matmul(psv2[:, k * 512:(k + 1) * 512],
                                     W(f"Wv2{l}"), hv1[:, ksl],
                                     start=True, stop=True)
                nc.vector.tensor_scalar_add(phirep[:, sl], psv2[:], Bia(f"bv2{l}"))
            for c in range(NGBL):
                pst = ps.tile([128, 128], F32, tag="ps4")
                nc.tensor.transpose(pst[:], phirep[:, c * 128:(c + 1) * 128], ident)
                nc.vector.tensor_copy(phiP[:, c:c + 1], pst[:, 0:1])
                nc.vector.tensor_copy(phiP[:, c + NGBL:c + NGBL + 1], pst[:, 64:65])

        def edge_phase(l):
            # ---- edge phase: software-pipelined over ug = 0..63 ----
            # unit ug: gmp = ug // 32, gb_l = (ug % 32) // 2, u = ug % 2
            # chunk c = gmp*16 + gb_l; EW partitions c*4..c*4+4
            NU = 64
            S = {}

            def unit_params(ug):
                gmp = ug // 32
                gb_l = (ug % 32) // 2
                u = ug % 2
                c = gmp * 16 + gb_l
                nb = gb_l * 128 + gmp * 64 + u * 32
                gmg = gmp * 2 + u
                return gmp, gb_l, u, c, nb, gmg

            def st0(ug):  # ps1: A/B identity passes + Wsc pass
                gmp, gb_l, u, c, nb, gmg = unit_params(ug)
                ps1 = ps.tile([128, 1024], F32, tag="ps1")
                rsc = S[("rsc", c)]
                for k in range(2):
                    ksl = slice(k * 512, (k + 1) * 512)
                    hi = h16[:, nb + k * 16:nb + (k + 1) * 16]
                    hi_bc = hi.unsqueeze(2).broadcast_to([128, 16, 32])
                    nc.tensor.matmul(ps1[:, ksl], W(f"Wi{l}"), hi_bc,
                                     start=True, stop=False)
                for k in range(2):
                    ksl = slice(k * 512, (k + 1) * 512)
                    hj_bc = h16[:, nb:nb + 32].unsqueeze(1).broadcast_to(
                        [128, 16, 32])
                    nc.tensor.matmul(ps1[:, ksl], W(f"Wj{l}"), hj_bc,
                                     start=False, stop=False)
                for k in range(2):
                    ksl = slice(k * 512, (k + 1) * 512)
                    nc.tensor.matmul(
                        ps1[:, ksl], W(f"Wsc{l}")[0:4, :],
                        rsc[:, u * 1024 + k * 512:u * 1024 + (k + 1) * 512],
                        start=False, stop=True)
                S[("ps1", ug)] = ps1

            def st1(ug):  # m1s silu
                m1s = eA.tile([128, 1024], BF16, tag="m1s")
                nc.scalar.activation(m1s[:], S.pop(("ps1", ug))[:], ACTF.Silu,
                                     bias=Bia(f"be1{l}"))
                S[("m1s", ug)] = m1s

            def st2(ug):  # ps2 = We2 @ m1s
                m1s = S.pop(("m1s", ug))
                ps2 = ps.tile([128, 1024], F32, tag="ps2")
                for k in range(2):
                    ksl = slice(k * 512, (k + 1) * 512)
                    nc.tensor.matmul(ps2[:, ksl], W(f"We2{l}"), m1s[:, ksl],
                                     start=True, stop=True)
                S[("ps2", ug)] = ps2

            def st3(ug):  # m_u silu
                m_u = eA.tile([128, 1024], BF16, tag="m_u")
                nc.scalar.activation(m_u[:], S.pop(("ps2", ug))[:], ACTF.Silu,
                                     bias=Bia(f"be2{l}"))
                S[("m_u", ug)] = m_u

            def st4(ug):  # magg/mdiag + ps3 = Wc1 @ m_u
                gmp, gb_l, u, c, nb, gmg = unit_params(ug)
                m_u = S.pop(("m_u", ug))
                if l < L - 1:  # h update is dead work on the last layer
                    # magg = sum_j m via a log-tree of adds on idle GpSimd
                    m3 = m_u[:].rearrange("p (i j) -> p i j", j=32)
                    tr = eA.tile([128, 512], BF16, tag="tr1")
                    t3 = tr[:].rearrange("p (i j) -> p i j", j=16)
                    nc.gpsimd.tensor_tensor(t3, m3[:, :, 0:16], m3[:, :, 16:32],
                                            op=AT.add)
                    w = 16
                    while w > 2:
                        nxt = eA.tile([128, 32 * (w // 2)], BF16, tag=f"tr{w}")
                        n3 = nxt[:].rearrange("p (i j) -> p i j", j=w // 2)
                        nc.gpsimd.tensor_tensor(n3, t3[:, :, 0:w // 2],
                                                t3[:, :, w // 2:w], op=AT.add)
                        t3 = n3
                        w //= 2
                    nc.gpsimd.tensor_tensor(magg[:, nb:nb + 32], t3[:, :, 0],
                                            t3[:, :, 1], op=AT.add)
                    nc.gpsimd.tensor_copy(mdiag[:, nb:nb + 32], m_u[:, 0:1024:33])
                ps3 = ps.tile([128, 1024], F32, tag="ps3")
                for k in range(2):
                    ksl = slice(k * 512, (k + 1) * 512)
                    nc.tensor.matmul(ps3[:, ksl], W(f"Wc1{l}"), m_u[:, ksl],
                                     start=True, stop=True)
                S[("ps3", ug)] = ps3

            def st5(ug):  # c1 silu
                c1 = eA.tile([128, 1024], BF16, tag="c1")
                nc.scalar.activation(c1[:], S.pop(("ps3", ug))[:], ACTF.Silu,
                                     bias=Bia(f"bc1{l}"))
                S[("c1", ug)] = c1

            def st6(ug):  # ps4 = Wc2 @ c1
                c1 = S.pop(("c1", ug))
                ps4 = ps.tile([128, 1024], F32, tag="ps4")
                for k in range(2):
                    ksl = slice(k * 512, (k + 1) * 512)
                    nc.tensor.matmul(ps4[:, ksl], W(f"Wc2{l}"), c1[:, ksl],
                                     start=True, stop=True)
                S[("ps4", ug)] = ps4

            def st7(ug):  # ssb = ps4 + bc2; scatter to smat
                gmp, gb_l, u, c, nb, gmg = unit_params(ug)
                ssb = eA.tile([128, 1024], F32, tag="ssb")
                nc.vector.tensor_scalar_add(ssb[:], S.pop(("ps4", ug))[:],
                                            Bia(f"bc2{l}"))
                pg = gmg * 32
                nc.sync.dma_start(
                    smat[pg:pg + 32, gb_l * 32:(gb_l + 1) * 32],
                    ssb[0:1, :].rearrange("p (i j) -> p i j", j=32))
                nc.sync.dma_start(
                    smat[pg:pg + 32, (gb_l + 16) * 32:(gb_l + 17) * 32],
                    ssb[64:65, :].rearrange("p (i j) -> p i j", j=32))

            def st_rsc(c):  # stage chunk c's EW rows at partition base 0
                rsc = eR.tile([4, 2048], BF16, tag="rsc")
                nc.sync.dma_start(rsc[:], EW[c * 4:c * 4 + 4, :])
                S[("rsc", c)] = rsc

            st_rsc(0)
            stages = (st0, st1, st2, st3, st4, st5, st6, st7)
            lags = (0, 0, 1, 1, 2, 2, 3, 3)
            for t in range(NU + 3):
                if t % 2 == 0 and t // 2 + 1 < NU // 2:
                    st_rsc(t // 2 + 1)
                for fn, lag in zip(stages, lags):
                    ug = t - lag
                    if 0 <= ug < NU:
                        fn(ug)
                if t % 2 == 1:
                    S.pop(("rsc", (t - 1) // 2), None)

        def h_update_phase(l):
            # ---- h update (split h); dead work on the last layer ----
            for u in range(NNODE // 1024 if l < L - 1 else 0):
                sl = slice(u * 1024, (u + 1) * 1024)
                psh = ps.tile([128, 1024], F32, tag="ps1")
                for k in range(2):
                    ksl = slice(u * 1024 + k * 512, u * 1024 + (k + 1) * 512)
                    osl = slice(k * 512, (k + 1) * 512)
                    nc.tensor.matmul(psh[:, osl], W(f"Wn1t{l}"),
                                     h16[:, ksl], start=True, stop=False)
                    nc.tensor.matmul(psh[:, osl], W(f"Wn1b{l}"),
                                     magg[:, ksl], start=False, stop=False)
                    nc.tensor.matmul(psh[:, osl], W(f"Wn1d{l}"),
                                     mdiag[:, ksl], start=False, stop=True)
                hn1 = eA.tile([128, 1024], BF16, tag="hn1")
                nc.scalar.activation(hn1[:], psh[:], ACTF.Silu, bias=Bia(f"bn1{l}"))
                psh2 = ps.tile([128, 1024], F32, tag="ps2")
                for k in range(2):
                    osl = slice(k * 512, (k + 1) * 512)
                    nc.tensor.matmul(psh2[:, osl], W(f"Wn2{l}"),
                                     hn1[:, osl], start=True, stop=True)
                nc.vector.scalar_tensor_tensor(
                    hF[:, sl], psh2[:], Bia(f"bn2{l}"), hF[:, sl],
                    op0=AT.add, op1=AT.add)
                h_shadows(sl)

        def matrix_phase(l):
            phiP = phiPA if l % 2 == 0 else phiPB
            # ---- matrix phase: t, u, agg, vel/loc update ----
            sq = mx.tile([128, 1024], F32, tag="mx_sq")
            nc.scalar.activation(sq[:], rad[:], ACTF.Sqrt)
            nc.gpsimd.tensor_scalar_add(sq[:], sq[:], 1.0)
            tm = mx.tile([128, 1024], F32, tag="mx_tm")
            nc.vector.reciprocal(tm[:], sq[:])
            um = mx.tile([128, 1024], F32, tag="mx_um")
            nc.gpsimd.tensor_tensor(um[:], tm[:], smat[:], op=AT.mult)
            for (dT, agg_out) in ((dx, "ax"), (dy, "ay")):
                w_ = mx.tile([128, 1024], F32, tag="mx_w" + agg_out)
                nc.vector.tensor_tensor(w_[:], um[:], dT[:], op=AT.mult)
                ag = mx.tile([128, NGB], F32, tag="mx_" + agg_out)
                nc.vector.tensor_reduce(
                    ag[:], w_[:].rearrange("p (gb j) -> p gb j", j=32),
                    axis=mybir.AxisListType.X, op=AT.add)
                vP = velx if agg_out == "ax" else vely
                tmp = mx.tile([128, NGB], F32, tag="mx_tmp")
                nc.vector.tensor_tensor(tmp[:], phiP[:], vP[:], op=AT.mult)
                nc.vector.scalar_tensor_tensor(vP[:], ag[:], 1.0 / DEG, tmp[:],
                                               op0=AT.mult, op1=AT.add)
            nc.vector.tensor_tensor(locx[:], locx[:], velx[:], op=AT.add)
            nc.vector.tensor_tensor(locy[:], locy[:], vely[:], op=AT.add)

        # ---- main layer loop: overlap matrix(l) with ab/node(l+1) ----
        ab_node_phase(0)
        for l in range(L):
            edge_phase(l)
            h_update_phase(l)
            if l < L - 1:
                ab_node_phase(l + 1)
            matrix_phase(l)
            if l < L - 1:
                radial_part(first=False)
                xi_ew_part()

        # ---- output: outP interleaved (gb, c) ----
        ov = outP[:].rearrange("p (gb c) -> p gb c", c=2)
        nc.vector.tensor_scalar(ov[:, :, 0], velx[:], scale0, mean0,
                                op0=AT.mult, op1=AT.add)
        nc.vector.tensor_scalar(ov[:, :, 1], vely[:], scale1, mean1,
                                op0=AT.mult, op1=AT.add)
        nc.sync.dma_start(out_d.ap(), outP[:])

    nc.compile()
    return nc


# ----------------------------------------------------------------------------
# Entry point
# ----------------------------------------------------------------------------

def kernel(**inputs):
    import concourse.mybir  # noqa: F401  (ensure env importable)
    from concourse.bass_utils import run_bass_kernel_spmd

    inp = {k: np.asarray(v) for k, v in inputs.items()}
    scale = np.asarray(inp["scale"], np.float32)
    mean = np.asarray(inp["mean"], np.float32)

    key = (float(scale[0]), float(scale[1]), float(mean[0]), float(mean[1]))
    if key not in _BUILD_CACHE:
        _BUILD_CACHE[key] = build(*key)
    nc = _BUILD_CACHE[key]

    in_maps = make_in_maps(inp)
    res = run_bass_kernel_spmd(nc, in_maps, list(range(NCORES)))
    outs = [_unarrange_output(res.results[c]["out"]) for c in range(NCORES)]
    return np.concatenate(outs, axis=0)


# revision 50
# speedup vs baseline: 1.0776x; 1.0776x over previous
"""Trainium2 Bass kernel for nn_DeterministicEgnnPolicy (EGNN message passing).

Strategy (per sharding hint): shard the 1024 independent 32-node graphs
across 8 NeuronCores (128 graphs/core). On each core the fully-connected
edge structure is computed densely as all-pairs 32x32 blocks:

- "feature-major" edge tensors [128 = 2 graph-halves x 64 features,
  (i, j)] drive the edge-MLP matmuls (bf16 operands, fp32 PSUM).
- Accuracy scheme (validated vs fp32 reference on CPU): h state is fp32
  master with bf16 hi/lo split shadows; per-node A = Wi.T h, B = Wj.T h are
  precomputed in split precision and broadcast into edge tensors via
  identity-stationary matmuls, so the big edge GEMMs see once-rounded bf16.
- Per-edge radial/edge_attr live in an "EW" tile [128 = (chunk, row),
  (u, i, j)] computed on-chip by DVE broadcast ops from per-graph coords
  (XI tiles, filled by selector matmuls) - no layout-conversion DMAs.
- Edge loop is software-pipelined (lag emission) across 4 PSUM stage banks.

Graph indexing on a core: g = gb*4 + gm, gb in [0,32), gm in [0,4).
half = gb//16 (feature partitions 64*half..64*half+63).
node free index (per half): n' = gb_l*128 + gm*32 + i, gb_l = gb%16.
chunk c = gmp*16 + gb_l, rows (0: rad h0, 1: ea h0, 2: rad h1, 3: ea h1).
"""

import numpy as np

N_AGENTS = 32
BATCH = 1024
H = 64
L = 4
INV = 16
DEG = float(N_AGENTS - 1)
NCORES = 8
G_CORE = BATCH // NCORES          # 128 graphs per core
NGB = G_CORE // 4                 # 32 gb blocks per core
NGBL = NGB // 2                   # 16 per half
NNODE = NGBL * 128                # 2048 node free dim (per half)
NODES_CORE = G_CORE * N_AGENTS    # 4096

_BUILD_CACHE = {}


# ----------------------------------------------------------------------------
# Host-side packing (pure layout permutation / weight arrangement)
# ----------------------------------------------------------------------------

def _bd(w):
    """64x64 block-diagonal lhsT [128,128] from w [64,64] (or [k,64])."""
    k = w.shape[0]
    out = np.zeros((128, 128), np.float32)
    out[0:k, 0:64] = w
    out[64:64 + k, 64:128] = w
    return out


def _bd_rep(wcol):
    """Replicating lhsT: out[64h+f, 64h+f'] = wcol[f] for all f'."""
    out = np.zeros((128, 128), np.float32)
    col = wcol.reshape(64, 1)
    out[0:64, 0:64] = np.repeat(col, 64, axis=1)
    out[64:128, 64:128] = np.repeat(col, 64, axis=1)
    return out


BNAMES = ["emb"]
for _l in range(L):
    BNAMES += [f"{nm}{_l}" for nm in
               ("Wi", "Wj", "Wsc", "We2", "Wc1",
                "Wc2", "Wv1", "Wv2", "Wn1t", "Wn1b", "Wn1d", "Wn2")]
NWB = len(BNAMES)
FNAMES = ["ident", "delta", "lsel"]


def _pack_weights(inp):
    """Build wpackb [128, NWB*128] (bf16), wpackf [128, 3*128] (f32),
    biaspack [128, NBIAS] (f32)."""
    import ml_dtypes

    btiles = {}

    def add(name, arr):
        t = np.zeros((128, 128), np.float32)
        t[:arr.shape[0], :arr.shape[1]] = arr
        btiles[name] = t

    emb = np.zeros((128, 128), np.float32)
    emb[0:INV, 0:64] = inp["emb_W"]
    emb[64:64 + INV, 64:128] = inp["emb_W"]
    add("emb", emb)

    for l in range(L):
        We1 = np.asarray(inp["We1"][l], np.float32)   # [130, 64]
        add(f"Wi{l}", _bd(We1[0:64]))
        add(f"Wj{l}", _bd(We1[64:128]))
        wsc = np.zeros((4, 128), np.float32)
        wsc[0, 0:64] = We1[128]      # radial, half0
        wsc[1, 0:64] = We1[129]      # edge_attr, half0
        wsc[2, 64:128] = We1[128]
        wsc[3, 64:128] = We1[129]
        add(f"Wsc{l}", wsc)
        add(f"We2{l}", _bd(inp["We2"][l]))
        add(f"Wc1{l}", _bd(inp["Wc1"][l]))
        add(f"Wc2{l}", _bd_rep(np.asarray(inp["Wc2"][l], np.float32)[:, 0]))
        add(f"Wv1{l}", _bd(inp["Wv1"][l]))
        add(f"Wv2{l}", _bd_rep(np.asarray(inp["Wv2"][l], np.float32)[:, 0]))
        Wn1 = np.asarray(inp["Wn1"][l], np.float32)   # [128, 64]
        add(f"Wn1t{l}", _bd(Wn1[0:64]))
        add(f"Wn1b{l}", _bd(Wn1[64:128]))
        add(f"Wn1d{l}", _bd(-Wn1[64:128]))
        add(f"Wn2{l}", _bd(inp["Wn2"][l]))

    wpackb = np.concatenate([btiles[n] for n in BNAMES], axis=1).astype(
        np.float16)

    ftiles = {}
    ftiles["ident"] = np.eye(128, dtype=np.float32)
    delta = np.zeros((128, 128), np.float32)
    for gm in range(4):
        delta[gm, gm * 32:(gm + 1) * 32] = 1.0
    ftiles["delta"] = delta
    # Lsel[k, gb_l*4 + row]: row0 -> cur x of gb_l (k=gb_l), row1 -> x0 of
    # gb_l (k=32+gb_l), row2 -> cur x of gb_l+16, row3 -> x0 of gb_l+16.
    lsel = np.zeros((128, 128), np.float32)
    for gb_l in range(NGBL):
        lsel[gb_l, gb_l * 4 + 0] = 1.0
        lsel[32 + gb_l, gb_l * 4 + 1] = 1.0
        lsel[gb_l + 16, gb_l * 4 + 2] = 1.0
        lsel[32 + gb_l + 16, gb_l * 4 + 3] = 1.0
    ftiles["lsel"] = lsel
    wpackf = np.concatenate([ftiles[n] for n in FNAMES], axis=1)

    bias_cols = []
    for l in range(L):
        for nm in ("be1", "be2", "bc1", "bv1", "bn1", "bn2"):
            bias_cols.append(np.tile(np.asarray(inp[nm][l]).reshape(-1), 2))
        for nm in ("bv2", "bc2"):
            bias_cols.append(np.full(128, float(np.asarray(inp[nm][l]).reshape(-1)[0]),
                                     np.float32))
    bias_cols.append(np.tile(np.asarray(inp["emb_b"]).reshape(-1), 2))
    biaspack = np.stack(bias_cols, axis=1).astype(np.float32)  # [128, NB]
    return wpackb, wpackf, biaspack


def _arrange_inputs(obs_slice):
    """Per-core obs slice [4096, 20] -> invT [128, 2048] fp16,
    locvel [128, 128] f32."""
    obs3 = obs_slice.reshape(NGB, 128, 20)          # [gb, (gm,i), col]
    invT = np.zeros((128, NNODE), np.float32)
    inv_half0 = obs3[0:NGBL, :, 0:INV]              # [16, 128, 16]
    inv_half1 = obs3[NGBL:NGB, :, 0:INV]
    invT[0:INV, :] = np.transpose(inv_half0, (2, 0, 1)).reshape(INV, NNODE)
    invT[64:64 + INV, :] = np.transpose(inv_half1, (2, 0, 1)).reshape(INV, NNODE)
    locvel = np.ascontiguousarray(
        np.transpose(obs3[:, :, INV:INV + 4], (1, 0, 2)).reshape(128, NGB * 4)
    ).astype(np.float32)
    return invT.astype(np.float16), locvel


def _unarrange_output(outP):
    """outP [128, 64] -> [4096, 2] (n = gb*128 + p)."""
    return np.ascontiguousarray(
        outP.reshape(128, NGB, 2).transpose(1, 0, 2).reshape(NODES_CORE, 2)
    )


def make_in_maps(inp):
    wpackb, wpackf, biaspack = _pack_weights(inp)
    obs = np.asarray(inp["obs"], np.float32)
    in_maps = []
    for c in range(NCORES):
        invT, locvel = _arrange_inputs(obs[c * NODES_CORE:(c + 1) * NODES_CORE])
        in_maps.append({"invT": invT, "locvel": locvel,
                        "wpackb": wpackb, "wpackf": wpackf,
                        "biaspack": biaspack})
    return in_maps


# ----------------------------------------------------------------------------
# Device kernel builder
# ----------------------------------------------------------------------------

def build(scale0, scale1, mean0, mean1):
    import concourse.bacc as bacc
    import concourse.tile as tile
    import concourse.mybir as mybir
    from contextlib import ExitStack

    F32 = mybir.dt.float32
    BF16 = mybir.dt.float16
    AT = mybir.AluOpType
    ACTF = mybir.ActivationFunctionType

    nc = bacc.Bacc("TRN2", target_bir_lowering=False, debug=False)

    invT_d = nc.dram_tensor("invT", [128, NNODE], BF16, kind="ExternalInput")
    locvel_d = nc.dram_tensor("locvel", [128, NGB * 4], F32, kind="ExternalInput")
    wpackb_d = nc.dram_tensor("wpackb", [128, NWB * 128], BF16, kind="ExternalInput")
    wpackf_d = nc.dram_tensor("wpackf", [128, len(FNAMES) * 128], F32,
                              kind="ExternalInput")
    NBIAS = 8 * L + 1
    bias_d = nc.dram_tensor("biaspack", [128, NBIAS], F32, kind="ExternalInput")
    out_d = nc.dram_tensor("out", [128, NGB * 2], F32, kind="ExternalOutput")

    widx = {n: i for i, n in enumerate(BNAMES)}
    fidx = {n: i for i, n in enumerate(FNAMES)}
    bidx = {}
    _bi = 0
    for l in range(L):
        for nm in ("be1", "be2", "bc1", "bv1", "bn1", "bn2", "bv2", "bc2"):
            bidx[f"{nm}{l}"] = _bi
            _bi += 1
    bidx["embb"] = _bi

    with tile.TileContext(nc) as tc, ExitStack() as ctx:
        st = ctx.enter_context(tc.tile_pool(name="static", bufs=1))
        eA = ctx.enter_context(tc.tile_pool(name="eA", bufs=2))
        eR = ctx.enter_context(tc.tile_pool(name="eR", bufs=3))
        mx = ctx.enter_context(tc.tile_pool(name="mx", bufs=1))
        ps = ctx.enter_context(tc.tile_pool(name="ps", bufs=1, space="PSUM"))

        # ---- static loads ----
        wsb = st.tile([128, NWB * 128], BF16)
        nc.sync.dma_start(wsb[:], wpackb_d.ap())
        wsf = st.tile([128, len(FNAMES) * 128], F32)
        nc.sync.dma_start(wsf[:], wpackf_d.ap())
        bsb = st.tile([128, NBIAS], F32)
        nc.sync.dma_start(bsb[:], bias_d.ap())
        invT = st.tile([128, NNODE], BF16)
        nc.sync.dma_start(invT[:], invT_d.ap())
        locvel = st.tile([128, NGB * 4], F32)
        nc.sync.dma_start(locvel[:], locvel_d.ap())

        def W(name):
            return wsb[:, widx[name] * 128:(widx[name] + 1) * 128]

        def Wf(name):
            return wsf[:, fidx[name] * 128:(fidx[name] + 1) * 128]

        def Bia(name):
            return bsb[:, bidx[name]:bidx[name] + 1]

        ident = Wf("ident")
        delta4 = Wf("delta")[0:4, :]
        lsel = Wf("lsel")[0:64, 0:64]

        # ---- persistent state ----
        hF = st.tile([128, NNODE], F32)
        h16 = st.tile([128, NNODE], BF16)
        magg = st.tile([128, NNODE], BF16)
        mdiag = st.tile([128, NNODE], BF16)
        smat = st.tile([128, 1024], F32)
        rad = st.tile([128, 1024], F32)
        dx = st.tile([128, 1024], F32)
        dy = st.tile([128, 1024], F32)
        EW = st.tile([128, 2048], BF16)
        XIx = st.tile([128, 64], F32)
        XIy = st.tile([128, 64], F32)
        LTx = st.tile([64, 128], F32)
        LTy = st.tile([64, 128], F32)
        locx = st.tile([128, NGB], F32)
        locy = st.tile([128, NGB], F32)
        velx = st.tile([128, NGB], F32)
        vely = st.tile([128, NGB], F32)
        phiPA = st.tile([128, NGB], F32)
        phiPB = st.tile([128, NGB], F32)
        hv1 = st.tile([128, NNODE], BF16)
        phirep = st.tile([128, NNODE], F32)
        T4x = st.tile([4, 1024], F32)
        T4y = st.tile([4, 1024], F32)
        outP = st.tile([128, NGB * 2], F32)

        lv = locvel[:].rearrange("p (gb c) -> p gb c", c=4)
        nc.vector.tensor_copy(locx[:], lv[:, :, 0])
        nc.vector.tensor_copy(locy[:], lv[:, :, 1])
        nc.vector.tensor_copy(velx[:], lv[:, :, 2])
        nc.vector.tensor_copy(vely[:], lv[:, :, 3])

        def heat(lhsT_ap, rhs_ap, n=14):
            hp = ps.tile([128, 512], F32, tag="ps2")
            for _ in range(n):
                nc.tensor.matmul(hp[:], lhsT_ap, rhs_ap, start=True, stop=True)

        # ---- embedding: h0 = inv @ emb_W + emb_b ----
        heat(W("emb"), invT[:, 0:512])
        for u in range(NNODE // 1024):
            pse = ps.tile([128, 1024], F32, tag="ps1")
            sl = slice(u * 1024, (u + 1) * 1024)
            for k in range(2):
                ksl = slice(u * 1024 + k * 512, u * 1024 + (k + 1) * 512)
                osl = slice(k * 512, (k + 1) * 512)
                nc.tensor.matmul(pse[:, osl], W("emb"), invT[:, ksl],
                                 start=True, stop=True)
            nc.vector.tensor_scalar_add(hF[:, sl], pse[:], Bia("embb"))

        def h_shadows(sl):
            nc.vector.tensor_copy(h16[:, sl], hF[:, sl])

        for u in range(2):
            h_shadows(slice(u * 1024, (u + 1) * 1024))

        def radial_part(first):
            """lxT/lyT -> LT tiles, T4s, dx, dy, rad from current locx/locy."""
            for (lP, LT) in ((locx, LTx), (locy, LTy)):
                pst = ps.tile([32, 128], F32, tag="ps3")
                nc.tensor.transpose(pst[:], lP[:], ident)
                nc.vector.tensor_copy(LT[0:32, :], pst[:])
                if first:
                    nc.vector.tensor_copy(LT[32:64, :], pst[:])
            for (LT, T4) in ((LTx, T4x), (LTy, T4y)):
                for gm in range(4):
                    nc.sync.dma_start(
                        T4[gm:gm + 1, :].rearrange("p (gb j) -> p gb j", j=32),
                        LT[0:32, gm * 32:(gm + 1) * 32])
            for (T4, lP, dT) in ((T4x, locx, dx), (T4y, locy, dy)):
                pss = ps.tile([128, 1024], F32, tag="ps2")
                for k in range(2):
                    nc.tensor.matmul(pss[:, k * 512:(k + 1) * 512], delta4,
                                     T4[:, k * 512:(k + 1) * 512],
                                     start=True, stop=True)
                bc = lP[:].unsqueeze(2).broadcast_to([128, NGB, 32])
                nc.vector.tensor_tensor(
                    dT[:].rearrange("p (gb j) -> p gb j", j=32), bc,
                    pss[:].rearrange("p (gb j) -> p gb j", j=32), op=AT.subtract)
            t2 = mx.tile([128, 1024], F32, tag="mx_t2")
            nc.vector.tensor_tensor(rad[:], dx[:], dx[:], op=AT.mult)
            nc.vector.tensor_tensor(t2[:], dy[:], dy[:], op=AT.mult)
            nc.vector.tensor_tensor(rad[:], rad[:], t2[:], op=AT.add)

        def xi_ew_part():
            """XI tiles from LT via selector matmuls, then EW = diff^2 sums."""
            for (LT, XI) in ((LTx, XIx), (LTy, XIy)):
                for gmp in range(2):
                    psxi = ps.tile([64, 64], F32, tag="ps4")
                    nc.tensor.matmul(psxi[:], lsel,
                                     LT[0:64, gmp * 64:(gmp + 1) * 64],
                                     start=True, stop=True)
                    nc.vector.tensor_copy(XI[gmp * 64:(gmp + 1) * 64, :], psxi[:])
            dxE = mx.tile([128, 2048], F32, tag="mx_dxE")
            dyE = mx.tile([128, 2048], F32, tag="mx_dyE")
            for (XI, dE) in ((XIx, dxE), (XIy, dyE)):
                x3 = XI[:].rearrange("p (u i) -> p u i", i=32)
                bi = x3.unsqueeze(3).broadcast_to([128, 2, 32, 32])
                bj = x3.unsqueeze(2).broadcast_to([128, 2, 32, 32])
                nc.vector.tensor_tensor(
                    dE[:].rearrange("p (u i j) -> p u i j", i=32, j=32),
                    bi, bj, op=AT.subtract)
            sq = mx.tile([128, 2048], F32, tag="mx_sqE")
            nc.vector.tensor_tensor(sq[:], dxE[:], dxE[:], op=AT.mult)
            sq2 = mx.tile([128, 2048], F32, tag="mx_sqE2")
            nc.vector.tensor_tensor(sq2[:], dyE[:], dyE[:], op=AT.mult)
            with nc.allow_low_precision(reason="bf16 EW radial"):
                nc.vector.tensor_tensor(EW[:], sq[:], sq2[:], op=AT.add)

        radial_part(first=True)
        xi_ew_part()

        def ab_node_phase(l):
            phiP = phiPA if l % 2 == 0 else phiPB
            # ---- node phase: phi = silu(h@Wv1+bv1)@Wv2 + bv2 -> phiP ----
            for u in range(NNODE // 1024):
                sl = slice(u * 1024, (u + 1) * 1024)
                psv = ps.tile([128, 1024], F32, tag="ps2")
                for k in range(2):
                    ksl = slice(u * 1024 + k * 512, u * 1024 + (k + 1) * 512)
                    osl = slice(k * 512, (k + 1) * 512)
                    nc.tensor.matmul(psv[:, osl], W(f"Wv1{l}"), h16[:, ksl],
                                     start=True, stop=True)
                nc.scalar.activation(hv1[:, sl], psv[:], ACTF.Silu,
                                     bias=Bia(f"bv1{l}"))
                psv2 = ps.tile([128, 1024], F32, tag="ps3")
                for k in range(2):
                    ksl = slice(u * 1024 + k * 512, u * 1024 + (k + 1) * 512)
                    nc.tensor.matmul(psv2[:, k * 512:(k + 1) * 512],
                                     W(f"Wv2{l}"), hv1[:, ksl],
                                     start=True, stop=True)
                nc.vector.tensor_scalar_add(phirep[:, sl], psv2[:], Bia(f"bv2{l}"))
            for c in range(NGBL):
                pst = ps.tile([128, 128], F32, tag="ps4")
                nc.tensor.transpose(pst[:], phirep[:, c * 128:(c + 1) * 128], ident)
                nc.vector.tensor_copy(phiP[:, c:c + 1], pst[:, 0:1])
                nc.vector.tensor_copy(phiP[:, c + NGBL:c + NGBL + 1], pst[:, 64:65])

        def edge_phase(l):
            # ---- edge phase: software-pipelined over ug = 0..63 ----
            # unit ug: gmp = ug // 32, gb_l = (ug % 32) // 2, u = ug % 2
            # chunk c = gmp*16 + gb_l; EW partitions c*4..c*4+4
            NU = 64
            S = {}

            def unit_params(ug):
                gmp = ug // 32
                gb_l = (ug % 32) // 2
                u = ug % 2
                c = gmp * 16 + gb_l
                nb = gb_l * 128 + gmp * 64 + u * 32
                gmg = gmp * 2 + u
                return gmp, gb_l, u, c, nb, gmg

            def st0(ug):  # ps1: A/B identity passes + Wsc pass
                gmp, gb_l, u, c, nb, gmg = unit_params(ug)
                ps1 = ps.tile([128, 1024], F32, tag="ps1")
                rsc = S[("rsc", c)]
                for k in range(2):
                    ksl = slice(k * 512, (k + 1) * 512)
                    hi = h16[:, nb + k * 16:nb + (k + 1) * 16]
                    hi_bc = hi.unsqueeze(2).broadcast_to([128, 16, 32])
                    nc.tensor.matmul(ps1[:, ksl], W(f"Wi{l}"), hi_bc,
                                     start=True, stop=False)
                for k in range(2):
                    ksl = slice(k * 512, (k + 1) * 512)
                    hj_bc = h16[:, nb:nb + 32].unsqueeze(1).broadcast_to(
                        [128, 16, 32])
                    nc.tensor.matmul(ps1[:, ksl], W(f"Wj{l}"), hj_bc,
                                     start=False, stop=False)
                for k in range(2):
                    ksl = slice(k * 512, (k + 1) * 512)
                    nc.tensor.matmul(
                        ps1[:, ksl], W(f"Wsc{l}")[0:4, :],
                        rsc[:, u * 1024 + k * 512:u * 1024 + (k + 1) * 512],
                        start=False, stop=True)
                S[("ps1", ug)] = ps1

            def st1(ug):  # m1s silu
                m1s = eA.tile([128, 1024], BF16, tag="m1s")
                nc.scalar.activation(m1s[:], S.pop(("ps1", ug))[:], ACTF.Silu,
                                     bias=Bia(f"be1{l}"))
                S[("m1s", ug)] = m1s

            def st2(ug):  # ps2 = We2 @ m1s
                m1s = S.pop(("m1s", ug))
                ps2 = ps.tile([128, 1024], F32, tag="ps2")
                for k in range(2):
                    ksl = slice(k * 512, (k + 1) * 512)
                    nc.tensor.matmul(ps2[:, ksl], W(f"We2{l}"), m1s[:, ksl],
                                     start=True, stop=True)
                S[("ps2", ug)] = ps2

            def st3(ug):  # m_u silu
                m_u = eA.tile([128, 1024], BF16, tag="m_u")
                nc.scalar.activation(m_u[:], S.pop(("ps2", ug))[:], ACTF.Silu,
                                     bias=Bia(f"be2{l}"))
                S[("m_u", ug)] = m_u

            def st4(ug):  # magg/mdiag + ps3 = Wc1 @ m_u
                gmp, gb_l, u, c, nb, gmg = unit_params(ug)
                m_u = S.pop(("m_u", ug))
                if l < L - 1:  # h update is dead work on the last layer
                    # magg = sum_j m via a log-tree of adds on idle GpSimd
                    m3 = m_u[:].rearrange("p (i j) -> p i j", j=32)
                    tr = eA.tile([128, 512], BF16, tag="tr1")
                    t3 = tr[:].rearrange("p (i j) -> p i j", j=16)
                    nc.gpsimd.tensor_tensor(t3, m3[:, :, 0:16], m3[:, :, 16:32],
                                            op=AT.add)
                    w = 16
                    while w > 2:
                        nxt = eA.tile([128, 32 * (w // 2)], BF16, tag=f"tr{w}")
                        n3 = nxt[:].rearrange("p (i j) -> p i j", j=w // 2)
                        nc.gpsimd.tensor_tensor(n3, t3[:, :, 0:w // 2],
                                                t3[:, :, w // 2:w], op=AT.add)
                        t3 = n3
                        w //= 2
                    nc.gpsimd.tensor_tensor(magg[:, nb:nb + 32], t3[:, :, 0],
                                            t3[:, :, 1], op=AT.add)
                    nc.gpsimd.tensor_copy(mdiag[:, nb:nb + 32], m_u[:, 0:1024:33])
                ps3 = ps.tile([128, 1024], F32, tag="ps3")
                for k in range(2):
                    ksl = slice(k * 512, (k + 1) * 512)
                    nc.tensor.matmul(ps3[:, ksl], W(f"Wc1{l}"), m_u[:, ksl],
                                     start=True, stop=True)
                S[("ps3", ug)] = ps3

            def st5(ug):  # c1 silu
                c1 = eA.tile([128, 1024], BF16, tag="c1")
                nc.scalar.activation(c1[:], S.pop(("ps3", ug))[:], ACTF.Silu,
                                     bias=Bia(f"bc1{l}"))
                S[("c1", ug)] = c1

            def st6(ug):  # ps4 = Wc2 @ c1
                c1 = S.pop(("c1", ug))
                ps4 = ps.tile([128, 1024], F32, tag="ps4")
                for k in range(2):
                    ksl = slice(k * 512, (k + 1) * 512)
                    nc.tensor.matmul(ps4[:, ksl], W(f"Wc2{l}"), c1[:, ksl],
                                     start=True, stop=True)
                S[("ps4", ug)] = ps4

            def st7(ug):  # ssb = ps4 + bc2; scatter to smat
                gmp, gb_l, u, c, nb, gmg = unit_params(ug)
                ssb = eA.tile([128, 1024], F32, tag="ssb")
                nc.vector.tensor_scalar_add(ssb[:], S.pop(("ps4", ug))[:],
                                            Bia(f"bc2{l}"))
                pg = gmg * 32
                nc.sync.dma_start(
                    smat[pg:pg + 32, gb_l * 32:(gb_l + 1) * 32],
                    ssb[0:1, :].rearrange("p (i j) -> p i j", j=32))
                nc.sync.dma_start(
                    smat[pg:pg + 32, (gb_l + 16) * 32:(gb_l + 17) * 32],
                    ssb[64:65, :].rearrange("p (i j) -> p i j", j=32))

            def st_rsc(c):  # stage chunk c's EW rows at partition base 0
                rsc = eR.tile([4, 2048], BF16, tag="rsc")
                nc.sync.dma_start(rsc[:], EW[c * 4:c * 4 + 4, :])
                S[("rsc", c)] = rsc

            st_rsc(0)
            stages = (st0, st1, st2, st3, st4, st5, st6, st7)
            lags = (0, 0, 1, 1, 2, 2, 3, 3)
            for t in range(NU + 3):
                if t % 2 == 0 and t // 2 + 1 < NU // 2:
                    st_rsc(t // 2 + 1)
                for fn, lag in zip(stages, lags):
                    ug = t - lag
                    if 0 <= ug < NU:
                        fn(ug)
                if t % 2 == 1:
                    S.pop(("rsc", (t - 1) // 2), None)

        def h_update_phase(l):
            # ---- h update (split h); dead work on the last layer ----
            for u in range(NNODE // 1024 if l < L - 1 else 0):
                sl = slice(u * 1024, (u + 1) * 1024)
                psh = ps.tile([128, 1024], F32, tag="ps1")
                for k in range(2):
                    ksl = slice(u * 1024 + k * 512, u * 1024 + (k + 1) * 512)
                    osl = slice(k * 512, (k + 1) * 512)
                    nc.tensor.matmul(psh[:, osl], W(f"Wn1t{l}"),
                                     h16[:, ksl], start=True, stop=False)
                    nc.tensor.matmul(psh[:, osl], W(f"Wn1b{l}"),
                                     magg[:, ksl], start=False, stop=False)
                    nc.tensor.matmul(psh[:, osl], W(f"Wn1d{l}"),
                                     mdiag[:, ksl], start=False, stop=True)
                hn1 = eA.tile([128, 1024], BF16, tag="hn1")
                nc.scalar.activation(hn1[:], psh[:], ACTF.Silu, bias=Bia(f"bn1{l}"))
                psh2 = ps.tile([128, 1024], F32, tag="ps2")
                for k in range(2):
                    osl = slice(k * 512, (k + 1) * 512)
                    nc.tensor.matmul(psh2[:, osl], W(f"Wn2{l}"),
                                     hn1[:, osl], start=True, stop=True)
                nc.vector.scalar_tensor_tensor(
                    hF[:, sl], psh2[:], Bia(f"bn2{l}"), hF[:, sl],
                    op0=AT.add, op1=AT.add)
                h_shadows(sl)

        def matrix_phase(l):
            phiP = phiPA if l % 2 == 0 else phiPB
            # ---- matrix phase: t, u, agg, vel/loc update ----
            sq = mx.tile([128, 1024], F32, tag="mx_sq")
            nc.scalar.activation(sq[:], rad[:], ACTF.Sqrt)
            nc.vector.tensor_scalar_add(sq[:], sq[:], 1.0)
            tm = mx.tile([128, 1024], F32, tag="mx_tm")
            nc.vector.reciprocal(tm[:], sq[:])
            um = mx.tile([128, 1024], F32, tag="mx_um")
            nc.vector.tensor_tensor(um[:], tm[:], smat[:], op=AT.mult)
            for (dT, agg_out) in ((dx, "ax"), (dy, "ay")):
                w_ = mx.tile([128, 1024], F32, tag="mx_w")
                nc.vector.tensor_tensor(w_[:], um[:], dT[:], op=AT.mult)
                ag = mx.tile([128, NGB], F32, tag="mx_" + agg_out)
                nc.vector.tensor_reduce(
                    ag[:], w_[:].rearrange("p (gb j) -> p gb j", j=32),
                    axis=mybir.AxisListType.X, op=AT.add)
                vP = velx if agg_out == "ax" else vely
                tmp = mx.tile([128, NGB], F32, tag="mx_tmp")
                nc.vector.tensor_tensor(tmp[:], phiP[:], vP[:], op=AT.mult)
                nc.vector.scalar_tensor_tensor(vP[:], ag[:], 1.0 / DEG, tmp[:],
                                               op0=AT.mult, op1=AT.add)
            nc.vector.tensor_tensor(locx[:], locx[:], velx[:], op=AT.add)
            nc.vector.tensor_tensor(locy[:], locy[:], vely[:], op=AT.add)

        # ---- main layer loop: overlap matrix(l) with ab/node(l+1) ----
        ab_node_phase(0)
        for l in range(L):
            edge_phase(l)
            h_update_phase(l)
            if l < L - 1:
                ab_node_phase(l + 1)
            matrix_phase(l)
            if l < L - 1:
                radial_part(first=False)
                xi_ew_part()

        # ---- output: outP interleaved (gb, c) ----
        ov = outP[:].rearrange("p (gb c) -> p gb c", c=2)
        nc.vector.tensor_scalar(ov[:, :, 0], velx[:], scale0, mean0,
                                op0=AT.mult, op1=AT.add)
        nc.vector.tensor_scalar(ov[:, :, 1], vely[:], scale1, mean1,
                                op0=AT.mult, op1=AT.add)
        nc.sync.dma_start(out_d.ap(), outP[:])

    nc.compile()
    return nc


# ----------------------------------------------------------------------------
# Entry point
# ----------------------------------------------------------------------------

def kernel(**inputs):
    import concourse.mybir  # noqa: F401  (ensure env importable)
    from concourse.bass_utils import run_bass_kernel_spmd

    inp = {k: np.asarray(v) for k, v in inputs.items()}
    scale = np.asarray(inp["scale"], np.float32)
    mean = np.asarray(inp["mean"], np.float32)

    key = (float(scale[0]), float(scale[1]), float(mean[0]), float(mean[1]))
    if key not in _BUILD_CACHE:
        _BUILD_CACHE[key] = build(*key)
    nc = _BUILD_CACHE[key]

    in_maps = make_in_maps(inp)
    res = run_bass_kernel_spmd(nc, in_maps, list(range(NCORES)))
    outs = [_unarrange_output(res.results[c]["out"]) for c in range(NCORES)]
    return np.concatenate(outs, axis=0)


# revision 53
# speedup vs baseline: 1.0916x; 1.0130x over previous
"""Trainium2 Bass kernel for nn_DeterministicEgnnPolicy (EGNN message passing).

Strategy (per sharding hint): shard the 1024 independent 32-node graphs
across 8 NeuronCores (128 graphs/core). On each core the fully-connected
edge structure is computed densely as all-pairs 32x32 blocks:

- "feature-major" edge tensors [128 = 2 graph-halves x 64 features,
  (i, j)] drive the edge-MLP matmuls (bf16 operands, fp32 PSUM).
- Accuracy scheme (validated vs fp32 reference on CPU): h state is fp32
  master with bf16 hi/lo split shadows; per-node A = Wi.T h, B = Wj.T h are
  precomputed in split precision and broadcast into edge tensors via
  identity-stationary matmuls, so the big edge GEMMs see once-rounded bf16.
- Per-edge radial/edge_attr live in an "EW" tile [128 = (chunk, row),
  (u, i, j)] computed on-chip by DVE broadcast ops from per-graph coords
  (XI tiles, filled by selector matmuls) - no layout-conversion DMAs.
- Edge loop is software-pipelined (lag emission) across 4 PSUM stage banks.

Graph indexing on a core: g = gb*4 + gm, gb in [0,32), gm in [0,4).
half = gb//16 (feature partitions 64*half..64*half+63).
node free index (per half): n' = gb_l*128 + gm*32 + i, gb_l = gb%16.
chunk c = gmp*16 + gb_l, rows (0: rad h0, 1: ea h0, 2: rad h1, 3: ea h1).
"""

import numpy as np

N_AGENTS = 32
BATCH = 1024
H = 64
L = 4
INV = 16
DEG = float(N_AGENTS - 1)
NCORES = 8
G_CORE = BATCH // NCORES          # 128 graphs per core
NGB = G_CORE // 4                 # 32 gb blocks per core
NGBL = NGB // 2                   # 16 per half
NNODE = NGBL * 128                # 2048 node free dim (per half)
NODES_CORE = G_CORE * N_AGENTS    # 4096

_BUILD_CACHE = {}


# ----------------------------------------------------------------------------
# Host-side packing (pure layout permutation / weight arrangement)
# ----------------------------------------------------------------------------

def _bd(w):
    """64x64 block-diagonal lhsT [128,128] from w [64,64] (or [k,64])."""
    k = w.shape[0]
    out = np.zeros((128, 128), np.float32)
    out[0:k, 0:64] = w
    out[64:64 + k, 64:128] = w
    return out


def _bd_rep(wcol):
    """Replicating lhsT: out[64h+f, 64h+f'] = wcol[f] for all f'."""
    out = np.zeros((128, 128), np.float32)
    col = wcol.reshape(64, 1)
    out[0:64, 0:64] = np.repeat(col, 64, axis=1)
    out[64:128, 64:128] = np.repeat(col, 64, axis=1)
    return out


BNAMES = ["emb"]
for _l in range(L):
    BNAMES += [f"{nm}{_l}" for nm in
               ("Wi", "Wj", "Wsc", "We2", "Wc1",
                "Wc2", "Wv1", "Wv2", "Wn1t", "Wn1b", "Wn1d", "Wn2")]
NWB = len(BNAMES)
FNAMES = ["ident", "delta", "lsel"]


def _pack_weights(inp):
    """Build wpackb [128, NWB*128] (bf16), wpackf [128, 3*128] (f32),
    biaspack [128, NBIAS] (f32)."""
    import ml_dtypes

    btiles = {}

    def add(name, arr):
        t = np.zeros((128, 128), np.float32)
        t[:arr.shape[0], :arr.shape[1]] = arr
        btiles[name] = t

    emb = np.zeros((128, 128), np.float32)
    emb[0:INV, 0:64] = inp["emb_W"]
    emb[64:64 + INV, 64:128] = inp["emb_W"]
    add("emb", emb)

    for l in range(L):
        We1 = np.asarray(inp["We1"][l], np.float32)   # [130, 64]
        add(f"Wi{l}", _bd(We1[0:64]))
        add(f"Wj{l}", _bd(We1[64:128]))
        wsc = np.zeros((4, 128), np.float32)
        wsc[0, 0:64] = We1[128]      # radial, half0
        wsc[1, 0:64] = We1[129]      # edge_attr, half0
        wsc[2, 64:128] = We1[128]
        wsc[3, 64:128] = We1[129]
        add(f"Wsc{l}", wsc)
        add(f"We2{l}", _bd(inp["We2"][l]))
        add(f"Wc1{l}", _bd(inp["Wc1"][l]))
        add(f"Wc2{l}", _bd_rep(np.asarray(inp["Wc2"][l], np.float32)[:, 0]))
        add(f"Wv1{l}", _bd(inp["Wv1"][l]))
        add(f"Wv2{l}", _bd_rep(np.asarray(inp["Wv2"][l], np.float32)[:, 0]))
        Wn1 = np.asarray(inp["Wn1"][l], np.float32)   # [128, 64]
        add(f"Wn1t{l}", _bd(Wn1[0:64]))
        add(f"Wn1b{l}", _bd(Wn1[64:128]))
        add(f"Wn1d{l}", _bd(-Wn1[64:128]))
        add(f"Wn2{l}", _bd(inp["Wn2"][l]))

    wpackb = np.concatenate([btiles[n] for n in BNAMES], axis=1).astype(
        np.float16)

    ftiles = {}
    ftiles["ident"] = np.eye(128, dtype=np.float32)
    delta = np.zeros((128, 128), np.float32)
    for gm in range(4):
        delta[gm, gm * 32:(gm + 1) * 32] = 1.0
    ftiles["delta"] = delta
    # Lsel[k, gb_l*4 + row]: row0 -> cur x of gb_l (k=gb_l), row1 -> x0 of
    # gb_l (k=32+gb_l), row2 -> cur x of gb_l+16, row3 -> x0 of gb_l+16.
    lsel = np.zeros((128, 128), np.float32)
    for gb_l in range(NGBL):
        lsel[gb_l, gb_l * 4 + 0] = 1.0
        lsel[32 + gb_l, gb_l * 4 + 1] = 1.0
        lsel[gb_l + 16, gb_l * 4 + 2] = 1.0
        lsel[32 + gb_l + 16, gb_l * 4 + 3] = 1.0
    ftiles["lsel"] = lsel
    wpackf = np.concatenate([ftiles[n] for n in FNAMES], axis=1)

    bias_cols = []
    for l in range(L):
        for nm in ("be1", "be2", "bc1", "bv1", "bn1", "bn2"):
            bias_cols.append(np.tile(np.asarray(inp[nm][l]).reshape(-1), 2))
        for nm in ("bv2", "bc2"):
            bias_cols.append(np.full(128, float(np.asarray(inp[nm][l]).reshape(-1)[0]),
                                     np.float32))
    bias_cols.append(np.tile(np.asarray(inp["emb_b"]).reshape(-1), 2))
    biaspack = np.stack(bias_cols, axis=1).astype(np.float32)  # [128, NB]
    return wpackb, wpackf, biaspack


def _arrange_inputs(obs_slice):
    """Per-core obs slice [4096, 20] -> invT [128, 2048] fp16,
    locvel [128, 128] f32."""
    obs3 = obs_slice.reshape(NGB, 128, 20)          # [gb, (gm,i), col]
    invT = np.zeros((128, NNODE), np.float32)
    inv_half0 = obs3[0:NGBL, :, 0:INV]              # [16, 128, 16]
    inv_half1 = obs3[NGBL:NGB, :, 0:INV]
    invT[0:INV, :] = np.transpose(inv_half0, (2, 0, 1)).reshape(INV, NNODE)
    invT[64:64 + INV, :] = np.transpose(inv_half1, (2, 0, 1)).reshape(INV, NNODE)
    locvel = np.ascontiguousarray(
        np.transpose(obs3[:, :, INV:INV + 4], (1, 0, 2)).reshape(128, NGB * 4)
    ).astype(np.float32)
    return invT.astype(np.float16), locvel


def _unarrange_output(outP):
    """outP [128, 64] -> [4096, 2] (n = gb*128 + p)."""
    return np.ascontiguousarray(
        outP.reshape(128, NGB, 2).transpose(1, 0, 2).reshape(NODES_CORE, 2)
    )


def make_in_maps(inp):
    wpackb, wpackf, biaspack = _pack_weights(inp)
    obs = np.asarray(inp["obs"], np.float32)
    in_maps = []
    for c in range(NCORES):
        invT, locvel = _arrange_inputs(obs[c * NODES_CORE:(c + 1) * NODES_CORE])
        in_maps.append({"invT": invT, "locvel": locvel,
                        "wpackb": wpackb, "wpackf": wpackf,
                        "biaspack": biaspack})
    return in_maps


# ----------------------------------------------------------------------------
# Device kernel builder
# ----------------------------------------------------------------------------

def build(scale0, scale1, mean0, mean1):
    import concourse.bacc as bacc
    import concourse.tile as tile
    import concourse.mybir as mybir
    from contextlib import ExitStack

    F32 = mybir.dt.float32
    BF16 = mybir.dt.float16
    AT = mybir.AluOpType
    ACTF = mybir.ActivationFunctionType

    nc = bacc.Bacc("TRN2", target_bir_lowering=False, debug=False)

    invT_d = nc.dram_tensor("invT", [128, NNODE], BF16, kind="ExternalInput")
    locvel_d = nc.dram_tensor("locvel", [128, NGB * 4], F32, kind="ExternalInput")
    wpackb_d = nc.dram_tensor("wpackb", [128, NWB * 128], BF16, kind="ExternalInput")
    wpackf_d = nc.dram_tensor("wpackf", [128, len(FNAMES) * 128], F32,
                              kind="ExternalInput")
    NBIAS = 8 * L + 1
    bias_d = nc.dram_tensor("biaspack", [128, NBIAS], F32, kind="ExternalInput")
    out_d = nc.dram_tensor("out", [128, NGB * 2], F32, kind="ExternalOutput")

    widx = {n: i for i, n in enumerate(BNAMES)}
    fidx = {n: i for i, n in enumerate(FNAMES)}
    bidx = {}
    _bi = 0
    for l in range(L):
        for nm in ("be1", "be2", "bc1", "bv1", "bn1", "bn2", "bv2", "bc2"):
            bidx[f"{nm}{l}"] = _bi
            _bi += 1
    bidx["embb"] = _bi

    with tile.TileContext(nc) as tc, ExitStack() as ctx:
        st = ctx.enter_context(tc.tile_pool(name="static", bufs=1))
        eA = ctx.enter_context(tc.tile_pool(name="eA", bufs=2))
        eR = ctx.enter_context(tc.tile_pool(name="eR", bufs=3))
        mx = ctx.enter_context(tc.tile_pool(name="mx", bufs=1))
        ps = ctx.enter_context(tc.tile_pool(name="ps", bufs=1, space="PSUM"))

        # ---- static loads ----
        wsb = st.tile([128, NWB * 128], BF16)
        nc.sync.dma_start(wsb[:], wpackb_d.ap())
        wsf = st.tile([128, len(FNAMES) * 128], F32)
        nc.sync.dma_start(wsf[:], wpackf_d.ap())
        bsb = st.tile([128, NBIAS], F32)
        nc.sync.dma_start(bsb[:], bias_d.ap())
        invT = st.tile([128, NNODE], BF16)
        nc.sync.dma_start(invT[:], invT_d.ap())
        locvel = st.tile([128, NGB * 4], F32)
        nc.sync.dma_start(locvel[:], locvel_d.ap())

        def W(name):
            return wsb[:, widx[name] * 128:(widx[name] + 1) * 128]

        def Wf(name):
            return wsf[:, fidx[name] * 128:(fidx[name] + 1) * 128]

        def Bia(name):
            return bsb[:, bidx[name]:bidx[name] + 1]

        ident = Wf("ident")
        delta4 = Wf("delta")[0:4, :]
        lsel = Wf("lsel")[0:64, 0:64]

        # ---- persistent state ----
        hF = st.tile([128, NNODE], F32)
        h16 = st.tile([128, NNODE], BF16)
        magg = st.tile([128, NNODE], BF16)
        mdiag = st.tile([128, NNODE], BF16)
        smat = st.tile([128, 1024], F32)
        rad = st.tile([128, 1024], F32)
        dx = st.tile([128, 1024], F32)
        dy = st.tile([128, 1024], F32)
        EW = st.tile([128, 2048], BF16)
        XIx = st.tile([128, 64], F32)
        XIy = st.tile([128, 64], F32)
        LTx = st.tile([64, 128], F32)
        LTy = st.tile([64, 128], F32)
        locx = st.tile([128, NGB], F32)
        locy = st.tile([128, NGB], F32)
        velx = st.tile([128, NGB], F32)
        vely = st.tile([128, NGB], F32)
        phiPA = st.tile([128, NGB], F32)
        phiPB = st.tile([128, NGB], F32)
        hv1 = st.tile([128, NNODE], BF16)
        phirep = st.tile([128, NNODE], F32)
        T4x = st.tile([4, 1024], F32)
        T4y = st.tile([4, 1024], F32)
        outP = st.tile([128, NGB * 2], F32)

        lv = locvel[:].rearrange("p (gb c) -> p gb c", c=4)
        nc.vector.tensor_copy(locx[:], lv[:, :, 0])
        nc.vector.tensor_copy(locy[:], lv[:, :, 1])
        nc.vector.tensor_copy(velx[:], lv[:, :, 2])
        nc.vector.tensor_copy(vely[:], lv[:, :, 3])

        def heat(lhsT_ap, rhs_ap, n=14):
            hp = ps.tile([128, 512], F32, tag="ps2")
            for _ in range(n):
                nc.tensor.matmul(hp[:], lhsT_ap, rhs_ap, start=True, stop=True)

        # ---- embedding: h0 = inv @ emb_W + emb_b ----
        heat(W("emb"), invT[:, 0:512])
        for u in range(NNODE // 1024):
            pse = ps.tile([128, 1024], F32, tag="ps1")
            sl = slice(u * 1024, (u + 1) * 1024)
            for k in range(2):
                ksl = slice(u * 1024 + k * 512, u * 1024 + (k + 1) * 512)
                osl = slice(k * 512, (k + 1) * 512)
                nc.tensor.matmul(pse[:, osl], W("emb"), invT[:, ksl],
                                 start=True, stop=True)
            nc.vector.tensor_scalar_add(hF[:, sl], pse[:], Bia("embb"))

        def h_shadows(sl):
            nc.vector.tensor_copy(h16[:, sl], hF[:, sl])

        for u in range(2):
            h_shadows(slice(u * 1024, (u + 1) * 1024))

        def lt_part(first):
            """PE-transpose current locx/locy into the LT tiles."""
            for (lP, LT) in ((locx, LTx), (locy, LTy)):
                pst = ps.tile([32, 128], F32, tag="ps3")
                nc.tensor.transpose(pst[:], lP[:], ident)
                nc.vector.tensor_copy(LT[0:32, :], pst[:])
                if first:
                    nc.vector.tensor_copy(LT[32:64, :], pst[:])

        def xi_ew_part():
            """XI tiles from LT via selector matmuls, then EW = diff^2 sums.
            This gates the next layer's edge phase - emit it before the
            deferrable radial_rest rebuild."""
            for (LT, XI) in ((LTx, XIx), (LTy, XIy)):
                for gmp in range(2):
                    psxi = ps.tile([64, 64], F32, tag="ps4")
                    nc.tensor.matmul(psxi[:], lsel,
                                     LT[0:64, gmp * 64:(gmp + 1) * 64],
                                     start=True, stop=True)
                    nc.vector.tensor_copy(XI[gmp * 64:(gmp + 1) * 64, :], psxi[:])
            dxE = mx.tile([128, 2048], BF16, tag="mx_dxE")
            dyE = mx.tile([128, 2048], BF16, tag="mx_dyE")
            for (XI, dE) in ((XIx, dxE), (XIy, dyE)):
                x3 = XI[:].rearrange("p (u i) -> p u i", i=32)
                bi = x3.unsqueeze(3).broadcast_to([128, 2, 32, 32])
                bj = x3.unsqueeze(2).broadcast_to([128, 2, 32, 32])
                with nc.allow_low_precision(reason="fp16 EW diffs"):
                    nc.vector.tensor_tensor(
                        dE[:].rearrange("p (u i j) -> p u i j", i=32, j=32),
                        bi, bj, op=AT.subtract)
            sq = mx.tile([128, 2048], BF16, tag="mx_sqE")
            nc.vector.tensor_tensor(sq[:], dxE[:], dxE[:], op=AT.mult)
            sq2 = mx.tile([128, 2048], BF16, tag="mx_sqE2")
            nc.vector.tensor_tensor(sq2[:], dyE[:], dyE[:], op=AT.mult)
            with nc.allow_low_precision(reason="fp16 EW radial"):
                nc.vector.tensor_tensor(EW[:], sq[:], sq2[:], op=AT.add)

        def radial_rest():
            """Matrix-layout dx/dy/rad rebuild - only needed by the NEXT
            matrix phase, so emitted after xi_ew_part."""
            for (LT, T4) in ((LTx, T4x), (LTy, T4y)):
                for gm in range(4):
                    nc.sync.dma_start(
                        T4[gm:gm + 1, :].rearrange("p (gb j) -> p gb j", j=32),
                        LT[0:32, gm * 32:(gm + 1) * 32])
            for (T4, lP, dT) in ((T4x, locx, dx), (T4y, locy, dy)):
                pss = ps.tile([128, 1024], F32, tag="ps2")
                for k in range(2):
                    nc.tensor.matmul(pss[:, k * 512:(k + 1) * 512], delta4,
                                     T4[:, k * 512:(k + 1) * 512],
                                     start=True, stop=True)
                bc = lP[:].unsqueeze(2).broadcast_to([128, NGB, 32])
                nc.vector.tensor_tensor(
                    dT[:].rearrange("p (gb j) -> p gb j", j=32), bc,
                    pss[:].rearrange("p (gb j) -> p gb j", j=32), op=AT.subtract)
            t2 = mx.tile([128, 1024], F32, tag="mx_t2")
            nc.vector.tensor_tensor(rad[:], dx[:], dx[:], op=AT.mult)
            nc.vector.tensor_tensor(t2[:], dy[:], dy[:], op=AT.mult)
            nc.vector.tensor_tensor(rad[:], rad[:], t2[:], op=AT.add)

        lt_part(first=True)
        xi_ew_part()
        radial_rest()

        def ab_node_phase(l):
            phiP = phiPA if l % 2 == 0 else phiPB
            # ---- node phase: phi = silu(h@Wv1+bv1)@Wv2 + bv2 -> phiP ----
            for u in range(NNODE // 1024):
                sl = slice(u * 1024, (u + 1) * 1024)
                psv = ps.tile([128, 1024], F32, tag="ps2")
                for k in range(2):
                    ksl = slice(u * 1024 + k * 512, u * 1024 + (k + 1) * 512)
                    osl = slice(k * 512, (k + 1) * 512)
                    nc.tensor.matmul(psv[:, osl], W(f"Wv1{l}"), h16[:, ksl],
                                     start=True, stop=True)
                nc.scalar.activation(hv1[:, sl], psv[:], ACTF.Silu,
                                     bias=Bia(f"bv1{l}"))
                psv2 = ps.tile([128, 1024], F32, tag="ps3")
                for k in range(2):
                    ksl = slice(u * 1024 + k * 512, u * 1024 + (k + 1) * 512)
                    nc.tensor.matmul(psv2[:, k * 512:(k + 1) * 512],
                                     W(f"Wv2{l}"), hv1[:, ksl],
                                     start=True, stop=True)
                nc.vector.tensor_scalar_add(phirep[:, sl], psv2[:], Bia(f"bv2{l}"))
            for c in range(NGBL):
                pst = ps.tile([128, 128], F32, tag="ps4")
                nc.tensor.transpose(pst[:], phirep[:, c * 128:(c + 1) * 128], ident)
                nc.vector.tensor_copy(phiP[:, c:c + 1], pst[:, 0:1])
                nc.vector.tensor_copy(phiP[:, c + NGBL:c + NGBL + 1], pst[:, 64:65])

        def edge_phase(l):
            # ---- edge phase: software-pipelined over ug = 0..63 ----
            # unit ug: gmp = ug // 32, gb_l = (ug % 32) // 2, u = ug % 2
            # chunk c = gmp*16 + gb_l; EW partitions c*4..c*4+4
            NU = 64
            S = {}

            def unit_params(ug):
                gmp = ug // 32
                gb_l = (ug % 32) // 2
                u = ug % 2
                c = gmp * 16 + gb_l
                nb = gb_l * 128 + gmp * 64 + u * 32
                gmg = gmp * 2 + u
                return gmp, gb_l, u, c, nb, gmg

            def st0(ug):  # ps1: A/B identity passes + Wsc pass
                gmp, gb_l, u, c, nb, gmg = unit_params(ug)
                ps1 = ps.tile([128, 1024], F32, tag="ps1")
                rsc = S[("rsc", c)]
                for k in range(2):
                    ksl = slice(k * 512, (k + 1) * 512)
                    hi = h16[:, nb + k * 16:nb + (k + 1) * 16]
                    hi_bc = hi.unsqueeze(2).broadcast_to([128, 16, 32])
                    nc.tensor.matmul(ps1[:, ksl], W(f"Wi{l}"), hi_bc,
                                     start=True, stop=False)
                for k in range(2):
                    ksl = slice(k * 512, (k + 1) * 512)
                    hj_bc = h16[:, nb:nb + 32].unsqueeze(1).broadcast_to(
                        [128, 16, 32])
                    nc.tensor.matmul(ps1[:, ksl], W(f"Wj{l}"), hj_bc,
                                     start=False, stop=False)
                for k in range(2):
                    ksl = slice(k * 512, (k + 1) * 512)
                    nc.tensor.matmul(
                        ps1[:, ksl], W(f"Wsc{l}")[0:4, :],
                        rsc[:, u * 1024 + k * 512:u * 1024 + (k + 1) * 512],
                        start=False, stop=True)
                S[("ps1", ug)] = ps1

            def st1(ug):  # m1s silu
                m1s = eA.tile([128, 1024], BF16, tag="m1s")
                nc.scalar.activation(m1s[:], S.pop(("ps1", ug))[:], ACTF.Silu,
                                     bias=Bia(f"be1{l}"))
                S[("m1s", ug)] = m1s

            def st2(ug):  # ps2 = We2 @ m1s
                m1s = S.pop(("m1s", ug))
                ps2 = ps.tile([128, 1024], F32, tag="ps2")
                for k in range(2):
                    ksl = slice(k * 512, (k + 1) * 512)
                    nc.tensor.matmul(ps2[:, ksl], W(f"We2{l}"), m1s[:, ksl],
                                     start=True, stop=True)
                S[("ps2", ug)] = ps2

            def st3(ug):  # m_u silu
                m_u = eA.tile([128, 1024], BF16, tag="m_u")
                nc.scalar.activation(m_u[:], S.pop(("ps2", ug))[:], ACTF.Silu,
                                     bias=Bia(f"be2{l}"))
                S[("m_u", ug)] = m_u

            def st4(ug):  # magg/mdiag + ps3 = Wc1 @ m_u
                gmp, gb_l, u, c, nb, gmg = unit_params(ug)
                m_u = S.pop(("m_u", ug))
                if l < L - 1:  # h update is dead work on the last layer
                    # magg = sum_j m via a log-tree of adds on idle GpSimd
                    m3 = m_u[:].rearrange("p (i j) -> p i j", j=32)
                    tr = eA.tile([128, 512], BF16, tag="tr1")
                    t3 = tr[:].rearrange("p (i j) -> p i j", j=16)
                    nc.gpsimd.tensor_tensor(t3, m3[:, :, 0:16], m3[:, :, 16:32],
                                            op=AT.add)
                    w = 16
                    while w > 2:
                        nxt = eA.tile([128, 32 * (w // 2)], BF16, tag=f"tr{w}")
                        n3 = nxt[:].rearrange("p (i j) -> p i j", j=w // 2)
                        nc.gpsimd.tensor_tensor(n3, t3[:, :, 0:w // 2],
                                                t3[:, :, w // 2:w], op=AT.add)
                        t3 = n3
                        w //= 2
                    nc.gpsimd.tensor_tensor(magg[:, nb:nb + 32], t3[:, :, 0],
                                            t3[:, :, 1], op=AT.add)
                    nc.gpsimd.tensor_copy(mdiag[:, nb:nb + 32], m_u[:, 0:1024:33])
                ps3 = ps.tile([128, 1024], F32, tag="ps3")
                for k in range(2):
                    ksl = slice(k * 512, (k + 1) * 512)
                    nc.tensor.matmul(ps3[:, ksl], W(f"Wc1{l}"), m_u[:, ksl],
                                     start=True, stop=True)
                S[("ps3", ug)] = ps3

            def st5(ug):  # c1 silu
                c1 = eA.tile([128, 1024], BF16, tag="c1")
                nc.scalar.activation(c1[:], S.pop(("ps3", ug))[:], ACTF.Silu,
                                     bias=Bia(f"bc1{l}"))
                S[("c1", ug)] = c1

            def st6(ug):  # ps4 = Wc2 @ c1
                c1 = S.pop(("c1", ug))
                ps4 = ps.tile([128, 1024], F32, tag="ps4")
                for k in range(2):
                    ksl = slice(k * 512, (k + 1) * 512)
                    nc.tensor.matmul(ps4[:, ksl], W(f"Wc2{l}"), c1[:, ksl],
                                     start=True, stop=True)
                S[("ps4", ug)] = ps4

            def st7(ug):  # ssb = ps4 + bc2; scatter to smat
                gmp, gb_l, u, c, nb, gmg = unit_params(ug)
                ssb = eA.tile([128, 1024], F32, tag="ssb")
                nc.vector.tensor_scalar_add(ssb[:], S.pop(("ps4", ug))[:],
                                            Bia(f"bc2{l}"))
                pg = gmg * 32
                nc.sync.dma_start(
                    smat[pg:pg + 32, gb_l * 32:(gb_l + 1) * 32],
                    ssb[0:1, :].rearrange("p (i j) -> p i j", j=32))
                nc.sync.dma_start(
                    smat[pg:pg + 32, (gb_l + 16) * 32:(gb_l + 17) * 32],
                    ssb[64:65, :].rearrange("p (i j) -> p i j", j=32))

            def st_rsc(c):  # stage chunk c's EW rows at partition base 0
                rsc = eR.tile([4, 2048], BF16, tag="rsc")
                nc.sync.dma_start(rsc[:], EW[c * 4:c * 4 + 4, :])
                S[("rsc", c)] = rsc

            st_rsc(0)
            stages = (st0, st1, st2, st3, st4, st5, st6, st7)
            lags = (0, 0, 1, 1, 2, 2, 3, 3)
            for t in range(NU + 3):
                if t % 2 == 0 and t // 2 + 1 < NU // 2:
                    st_rsc(t // 2 + 1)
                for fn, lag in zip(stages, lags):
                    ug = t - lag
                    if 0 <= ug < NU:
                        fn(ug)
                if t % 2 == 1:
                    S.pop(("rsc", (t - 1) // 2), None)

        def h_update_phase(l):
            # ---- h update (split h); dead work on the last layer ----
            for u in range(NNODE // 1024 if l < L - 1 else 0):
                sl = slice(u * 1024, (u + 1) * 1024)
                psh = ps.tile([128, 1024], F32, tag="ps1")
                for k in range(2):
                    ksl = slice(u * 1024 + k * 512, u * 1024 + (k + 1) * 512)
                    osl = slice(k * 512, (k + 1) * 512)
                    nc.tensor.matmul(psh[:, osl], W(f"Wn1t{l}"),
                                     h16[:, ksl], start=True, stop=False)
                    nc.tensor.matmul(psh[:, osl], W(f"Wn1b{l}"),
                                     magg[:, ksl], start=False, stop=False)
                    nc.tensor.matmul(psh[:, osl], W(f"Wn1d{l}"),
                                     mdiag[:, ksl], start=False, stop=True)
                hn1 = eA.tile([128, 1024], BF16, tag="hn1")
                nc.scalar.activation(hn1[:], psh[:], ACTF.Silu, bias=Bia(f"bn1{l}"))
                psh2 = ps.tile([128, 1024], F32, tag="ps2")
                for k in range(2):
                    osl = slice(k * 512, (k + 1) * 512)
                    nc.tensor.matmul(psh2[:, osl], W(f"Wn2{l}"),
                                     hn1[:, osl], start=True, stop=True)
                nc.vector.scalar_tensor_tensor(
                    hF[:, sl], psh2[:], Bia(f"bn2{l}"), hF[:, sl],
                    op0=AT.add, op1=AT.add)
                h_shadows(sl)

        def matrix_phase(l):
            phiP = phiPA if l % 2 == 0 else phiPB
            # ---- matrix phase: t, u, agg, vel/loc update ----
            sq = mx.tile([128, 1024], F32, tag="mx_sq")
            nc.scalar.activation(sq[:], rad[:], ACTF.Sqrt)
            nc.vector.tensor_scalar_add(sq[:], sq[:], 1.0)
            tm = mx.tile([128, 1024], F32, tag="mx_tm")
            nc.vector.reciprocal(tm[:], sq[:])
            um = mx.tile([128, 1024], F32, tag="mx_um")
            nc.vector.tensor_tensor(um[:], tm[:], smat[:], op=AT.mult)
            for (dT, agg_out) in ((dx, "ax"), (dy, "ay")):
                w_ = mx.tile([128, 1024], F32, tag="mx_w")
                nc.vector.tensor_tensor(w_[:], um[:], dT[:], op=AT.mult)
                ag = mx.tile([128, NGB], F32, tag="mx_" + agg_out)
                nc.vector.tensor_reduce(
                    ag[:], w_[:].rearrange("p (gb j) -> p gb j", j=32),
                    axis=mybir.AxisListType.X, op=AT.add)
                vP = velx if agg_out == "ax" else vely
                lP = locx if agg_out == "ax" else locy
                tmp = mx.tile([128, NGB], F32, tag="mx_tmp")
                nc.vector.tensor_tensor(tmp[:], phiP[:], vP[:], op=AT.mult)
                nc.vector.scalar_tensor_tensor(vP[:], ag[:], 1.0 / DEG, tmp[:],
                                               op0=AT.mult, op1=AT.add)
                nc.vector.tensor_tensor(lP[:], lP[:], vP[:], op=AT.add)

        # ---- main layer loop: overlap matrix(l) with ab/node(l+1) ----
        ab_node_phase(0)
        for l in range(L):
            edge_phase(l)
            h_update_phase(l)
            if l < L - 1:
                ab_node_phase(l + 1)
            matrix_phase(l)
            if l < L - 1:
                lt_part(first=False)
                xi_ew_part()
                radial_rest()

        # ---- output: outP interleaved (gb, c) ----
        ov = outP[:].rearrange("p (gb c) -> p gb c", c=2)
        nc.vector.tensor_scalar(ov[:, :, 0], velx[:], scale0, mean0,
                                op0=AT.mult, op1=AT.add)
        nc.vector.tensor_scalar(ov[:, :, 1], vely[:], scale1, mean1,
                                op0=AT.mult, op1=AT.add)
        nc.sync.dma_start(out_d.ap(), outP[:])

    nc.compile()
    return nc


# ----------------------------------------------------------------------------
# Entry point
# ----------------------------------------------------------------------------

def kernel(**inputs):
    import concourse.mybir  # noqa: F401  (ensure env importable)
    from concourse.bass_utils import run_bass_kernel_spmd

    inp = {k: np.asarray(v) for k, v in inputs.items()}
    scale = np.asarray(inp["scale"], np.float32)
    mean = np.asarray(inp["mean"], np.float32)

    key = (float(scale[0]), float(scale[1]), float(mean[0]), float(mean[1]))
    if key not in _BUILD_CACHE:
        _BUILD_CACHE[key] = build(*key)
    nc = _BUILD_CACHE[key]

    in_maps = make_in_maps(inp)
    res = run_bass_kernel_spmd(nc, in_maps, list(range(NCORES)))
    outs = [_unarrange_output(res.results[c]["out"]) for c in range(NCORES)]
    return np.concatenate(outs, axis=0)
